# revision 11
# baseline (speedup 1.0000x reference)
"""CompositePerturbation Trainium2 kernel (v4 — pipelined, packed params).

Pipeline per sample (batch sharded 4-per-core across 8 cores):
  1. Separable 25-tap blur as two banded-matmul passes with the BAND as the
     moving operand, streaming only its ~146 nonzero columns per 128-block:
       pass1: Y^T[w,h] = sum_k X_k[h_in,w]^T-free  x  bandV[h_in, h_out-narrow]
       pass2: Z[h,w']  = sum_j Y^T_j[w,h]-cols     x  bandH[w,  w'-narrow]
     All matmuls are narrow (32-aligned ranges): the k=0 matmul carries
     start=True, marking the whole 2KB zero-region pending; later matmuls
     accumulate into already-written bytes and overwrite still-pending ones
     (per-byte has_written semantics, verified on HW). x/band are bf16.
  2. Glare + occlusion: single K=2 bf16 rank-2 matmul (gy|-BIG*ro x
     gx|co) injected first (full width) into each pass-2 PSUM tile.
  3. Rain: count = K=60 matmul of exact 0/1 fp8 streak masks; decay
     D = exp(L*count) on ScalarE with per-partition scale L (fp32 exact).
  4. t = clamp01(z)*D + 1 - D  (custom DVE op, PSUM source) -> bf16.
  5. Salt/pepper as a host-precomputed exact trit mask m in {0,1,2} (fp8):
     out = min(max(t, m-1), m) as a single fused custom DVE op.

v4 vs v3:
  - params packed: band+scal one bf16 DMA, rn+gvt one fp8 DMA (bitcast APs)
  - x(c0) DMA issued first so pass1 starts ~6us earlier
  - junk matmuls dropped; rain matmuls sit between pass1(c0) and pass2(c0)
  - salt/pepper per-u tile (per-(c,u) on the last sample to shrink the tail)
  - 4 pass-2 PSUM banks
"""

import numpy as np

B, C, H, W = 32, 3, 512, 512
NCORES = 8
BPC = B // NCORES  # samples per core
KS = 25
HALF = KS // 2  # 12
P = 128
NT = H // P  # 4 partition tiles per image
BIGNEG = 100.0
CW = 3 * W  # 1536
BW = 192  # slim band tile width (covers offsets -32..160 of each k-block)

_CACHE = {}

# Narrow output ranges per contraction block k, superset of the true
# support [k*128-12, k*128+140). Offsets MUST stay 32-aligned: unaligned
# partial-PSUM matmul writes hard-wedge the device (NRT_EXEC_UNIT_UNRECOVERABLE).
def _nrange(k):
    return max(0, k * P - 32), min(512, k * P + P + 32)


def _host_params(x, sigma_u, glare_u, occ_u, rain_u, rain_n_u, rain_alpha_u,
                 noise_u, noise_amt_u, apply_flags):
    import ml_dtypes
    f32 = np.float32
    bf16 = ml_dtypes.bfloat16
    fp8 = ml_dtypes.float8_e4m3
    flags = apply_flags.astype(np.int64)

    # ---- blur band tensor [B, 128, 192]: band[r, t] = f(t - 32 - r) ----
    sigma = 1.0 + 3.0 * sigma_u.astype(np.float64)
    coords = np.arange(KS, dtype=np.float64) - HALF
    g = np.exp(-coords[None, :] ** 2 / (2.0 * sigma[:, None] ** 2))
    g = (g / g.sum(axis=1, keepdims=True)).astype(f32)  # [B, 25]
    band = np.zeros((B, P, BW), dtype=f32)
    r = np.arange(P)[:, None]
    t = np.arange(BW)[None, :]
    d = t - 32 - r
    inband = np.abs(d) <= HALF
    for b in range(B):
        if flags[b, 0] > 0:
            vals = np.zeros((P, BW), dtype=f32)
            vals[inband] = g[b][(d[inband] + HALF)]
            band[b] = vals
        else:
            band[b] = (d == 0).astype(f32)
    band = band.astype(bf16)

    # ---- glare/occl rank-2 tensor gvt [B, 2, 1024] ----
    yy = np.arange(H, dtype=f32)
    xx = np.arange(W, dtype=f32)
    inten = 0.4 + 0.5 * glare_u[:, 0]
    rx = (0.1 + 0.25 * glare_u[:, 1]) * W / 2
    ry = (0.1 + 0.25 * glare_u[:, 2]) * H / 2
    cx = (0.2 + 0.6 * glare_u[:, 3]) * W
    cy = (0.2 + 0.6 * glare_u[:, 4]) * H
    gy = np.exp(-((yy[None, :] - cy[:, None]) / ry[:, None]) ** 2)
    gx = np.exp(-((xx[None, :] - cx[:, None]) / rx[:, None]) ** 2)
    gy = gy * inten[:, None] * (flags[:, 1] > 0)[:, None]

    ph = np.floor(H * (0.1 + 0.3 * occ_u[:, 0]))
    pw = np.floor(W * (0.1 + 0.3 * occ_u[:, 1]))
    y0 = np.floor(occ_u[:, 2] * (H - ph))
    x0 = np.floor(occ_u[:, 3] * (W - pw))
    ro = ((yy[None, :] >= y0[:, None]) & (yy[None, :] < (y0 + ph)[:, None]))
    co = ((xx[None, :] >= x0[:, None]) & (xx[None, :] < (x0 + pw)[:, None]))
    ro = ro & (flags[:, 2] > 0)[:, None]

    gvt = np.zeros((B, 2, 1024), dtype=f32)
    gvt[:, 0, :512] = gy
    gvt[:, 1, :512] = (-BIGNEG) * ro
    gvt[:, 0, 512:] = gx
    gvt[:, 1, 512:] = co
    gvt = gvt.astype(bf16)

    # ---- rain streak masks (exact 0/1 in fp8), L via activation scale ----
    S = rain_u.shape[1]
    n = np.floor(20.0 + 41.0 * rain_n_u)
    a = 0.15 + 0.35 * rain_alpha_u
    L = np.where(flags[:, 3] > 0, np.log(1.0 - a), 0.0).astype(f32)
    xc = np.floor(rain_u[:, :, 0] * W)
    y0s = np.floor(rain_u[:, :, 1] * (H // 2))
    y1s = (H // 2) + np.floor(rain_u[:, :, 2] * (H // 2))
    hh = np.arange(H, dtype=f32)[None, None, :]
    ww = np.arange(W, dtype=f32)[None, None, :]
    rowm = ((hh >= y0s[:, :, None]) & (hh < y1s[:, :, None]))
    colm = ((ww >= xc[:, :, None] - 1) & (ww <= xc[:, :, None]))
    active = (np.arange(S)[None, :] < n[:, None])
    rn = np.concatenate([rowm & active[:, :, None], colm], axis=2)
    rn = rn.astype(fp8)  # [B, 60, 1024]

    scal = np.zeros((B, P, 2), dtype=f32)
    scal[:, :, 0] = L[:, None]

    # ---- packed param tensors ----
    # pb [B, 128, 196] bf16: cols 0:192 band, cols 192:196 scal (f32 bytes)
    pb = np.zeros((B, P, 196), dtype=bf16)
    pb[:, :, 0:BW] = band
    pb[:, :, BW:BW + 4] = scal.view(bf16)
    # rg [B, 66, 2048] fp8: rows 0:60 cols 0:1024 rn, rows 64:66 gvt bytes
    # (gvt must sit at a 32-aligned partition for matmul tile_position)
    rg = np.zeros((B, 66, 2048), dtype=fp8)
    rg[:, 0:60, 0:1024] = rn
    rg[:, 64:66, :] = gvt.view(fp8)

    # ---- salt/pepper trit mask m in {0,1,2}, exact fp32 compares ----
    amount = (0.01 + 0.07 * noise_amt_u)[:, None, None, None]
    f4 = (flags[:, 4] > 0)[:, None, None, None]
    lo = np.where(f4, amount / 2, 0.0)
    hi = np.where(f4, 1.0 - amount / 2, 2.0)
    trit = (1.0 + (noise_u > hi) - (noise_u < lo)).astype(fp8)
    m = np.ascontiguousarray(
        trit.reshape(B, C, NT, P, W).transpose(0, 3, 2, 1, 4)
    ).reshape(B, P, NT * CW)

    # ---- x packed [B, C, 128, NT*W] bf16 (channel-contiguous) ----
    xp = np.ascontiguousarray(
        x.astype(bf16).reshape(B, C, NT, P, W).transpose(0, 1, 3, 2, 4)
    ).reshape(B, C, P, NT * W)

    return xp, m, pb, rg


def _register_dve_ops():
    """Register the fused rain/salt-pepper custom DVE ops.

    CPERT_RAIN: out = clamp01(in0) * in1 + 1 - in1
      (in0 = blur+glare-BIG*occl PSUM, in1 = rain decay D)
    CPERT_SP2:  out = min(max(in1, in0 - 1), in0)
      (in0 = trit mask m, in1 = t)
    """
    from concourse import dve_ops
    from concourse.dve_spec import (
        Spec, Src0, Src1, Zero, One, maxx, minn, lower, _has_src1,
    )
    from concourse.dve_uop import DveOpSpec
    import numpy as np

    if "CPERT_RAIN_ANT" in dve_ops._SUB_OPCODE_FOR_NAME:
        return (dve_ops._BY_NAME_CPERT["CPERT_RAIN_ANT"],
                dve_ops._BY_NAME_CPERT["CPERT_SP2_ANT"])

    def make(name, spec):
        row = dve_ops._CUSTOM_DVE_ROW_BASE + len(dve_ops.OPS)
        assert row < 0x20
        shas = {}
        for ver in ("v3", "v4"):
            tmp = DveOpSpec(name=name, opcode=row, uops=lower(spec, ver=ver),
                            rd1_en=_has_src1(spec))
            shas[ver] = tmp.sha(ver)
        op = dve_ops.DveOp(name, spec, False, shas)
        dve_ops._SUB_OPCODE_FOR_NAME[name] = row
        dve_ops.OPS.append(op)
        dve_ops.CUSTOM_DVE_SPECS[name] = spec
        return op

    rain_spec = Spec(
        body=maxx(minn(Src0, One), Zero) * Src1 + One - Src1,
        reference=lambda in0, in1, s0, s1, imm2: (
            np.clip(in0, 0.0, 1.0).astype(np.float32) * in1 + 1.0 - in1
        ).astype(np.float32),
    )
    sp_spec = Spec(
        body=minn(maxx(Src1, Src0 - One), Src0),
        reference=lambda in0, in1, s0, s1, imm2: np.minimum(
            np.maximum(in1, in0 - 1.0), in0).astype(np.float32),
    )
    rain_op = make("CPERT_RAIN_ANT", rain_spec)
    sp_op = make("CPERT_SP2_ANT", sp_spec)
    dve_ops._BY_NAME_CPERT = {"CPERT_RAIN_ANT": rain_op,
                              "CPERT_SP2_ANT": sp_op}
    return rain_op, sp_op


def _build_module():
    import concourse.bacc as bacc
    import concourse.mybir as mybir
    from concourse.tile import TileContext

    f32 = mybir.dt.float32
    bf16 = mybir.dt.bfloat16
    fp8 = mybir.dt.float8e4
    AF = mybir.ActivationFunctionType

    RAIN_OP, SP_OP = _register_dve_ops()

    nc = bacc.Bacc("TRN2", target_bir_lowering=False, debug=False,
                   num_devices=NCORES)
    x_d = nc.declare_dram_parameter("x", [BPC, C, P, NT * W], bf16, isOutput=False)
    m_d = nc.declare_dram_parameter("m", [BPC, P, NT * CW], fp8, isOutput=False)
    pb_d = nc.declare_dram_parameter("pb", [BPC, P, 196], bf16, isOutput=False)
    rg_d = nc.declare_dram_parameter("rg", [BPC, 66, 2048], fp8, isOutput=False)
    out_d = nc.declare_dram_parameter("out", [BPC, NT, P, CW], bf16, isOutput=True)

    with TileContext(nc) as tc:
        with (
            tc.tile_pool(name="params", bufs=2) as ppool,
            tc.tile_pool(name="xin", bufs=5) as xpool,
            tc.tile_pool(name="ytsb", bufs=2) as ytpool,
            tc.tile_pool(name="tcat", bufs=8) as tpool,
            tc.tile_pool(name="ncat", bufs=2) as npool,
            tc.tile_pool(name="dd", bufs=8) as dpool,
            tc.tile_pool(name="oc", bufs=8) as opool,
            tc.tile_pool(name="yps", bufs=2, space="PSUM") as ypsum,
            tc.tile_pool(name="zps", bufs=4, space="PSUM") as zpsum,
            tc.tile_pool(name="rps", bufs=2, space="PSUM") as rpsum,
        ):
            warmed = False
            for b in range(BPC):
                last = (b == BPC - 1)
                # x(c0) first: it gates pass 1 of this sample
                xq = []
                xt0 = xpool.tile([P, NT * W], bf16, tag="x", name=f"x{b}_0")
                nc.sync.dma_start(out=xt0[:], in_=x_d[b, 0])
                xq.append(xt0)

                pbt = ppool.tile([P, 196], bf16, tag="pb")
                nc.sync.dma_start(out=pbt[:], in_=pb_d[b])
                bandb = pbt[:, 0:BW]
                sc = pbt[:, BW:BW + 4].bitcast(f32)

                rgt = ppool.tile([66, 2048], fp8, tag="rg")
                nc.sync.dma_start(out=rgt[:], in_=rg_d[b])
                rn = rgt[0:60, 0:1024]
                gvt = rgt[64:66, :].bitcast(bf16)

                ncat = npool.tile([P, NT * CW], fp8, tag="n")
                nc.sync.dma_start(out=ncat[:], in_=m_d[b])

                for c in range(1, C):
                    xt = xpool.tile([P, NT * W], bf16, tag="x", name=f"x{b}_{c}")
                    nc.sync.dma_start(out=xt[:], in_=x_d[b, c])
                    xq.append(xt)

                if not warmed:
                    # dummy Exp absorbs the ACT table load + bias-const dep
                    warm = ppool.tile([P, 2], f32, tag="warm")
                    nc.scalar.activation(warm[:, 0:1], sc[:, 1:2], AF.Exp)
                    warmed = True

                tcat = [tpool.tile([P, CW], bf16, tag="t", name=f"tc{b}_{u}")
                        for u in range(NT)]
                ocat = [opool.tile([P, CW], bf16, tag="o", name=f"oc{b}_{u}")
                        for u in range(NT)]
                D_t = [None] * NT

                for c in range(C):
                    # ---- pass 1: Y^T tiles, band moving w/ narrow columns ----
                    ytsb = ytpool.tile([P, NT * W], bf16, tag="yt")
                    for i in range(NT):
                        psY = ypsum.tile([P, W], f32, tag="psY")
                        # all-narrow: k=0 start=True marks the bank pending;
                        # later matmuls accumulate written bytes, overwrite
                        # still-pending ones (per-byte has_written)
                        for k in range(NT):
                            c0, c1 = _nrange(k)
                            nc.tensor.matmul(
                                psY[:, c0:c1],
                                lhsT=xq[c][:, k * W + i * P: k * W + (i + 1) * P],
                                rhs=bandb[:, c0 - k * P + 32: c1 - k * P + 32],
                                start=(k == 0), stop=(k == NT - 1))
                        nc.scalar.copy(ytsb[:, i * W:(i + 1) * W], psY[:])

                    if c == 0:
                        # ---- rain decay D[u] = exp(L * count), after pass1
                        # issue so the PE isn't stalled waiting on rn ----
                        for u in range(NT):
                            psA = rpsum.tile([P, W], f32, tag="psA")
                            nc.tensor.matmul(psA[:],
                                             lhsT=rn[0:60, u * P:(u + 1) * P],
                                             rhs=rn[0:60, 512:1024],
                                             start=True, stop=True)
                            dt_ = dpool.tile([P, W], f32, tag="D",
                                             name=f"D{b}_{u}")
                            nc.scalar.activation(dt_[:], psA[:], AF.Exp,
                                                 bias=sc[:, 1:2],
                                                 scale=sc[:, 0:1])
                            D_t[u] = dt_

                    # ---- pass 2 + glare/occl inject + rain + salt/pepper ----
                    for u in range(NT):
                        psZ = zpsum.tile([P, W], f32, tag="psZ")
                        nc.tensor.matmul(psZ[:], lhsT=gvt[:, u * P:(u + 1) * P],
                                         rhs=gvt[:, 512:1024],
                                         start=True, stop=False)
                        for j in range(NT):
                            c0, c1 = _nrange(j)
                            nc.tensor.matmul(
                                psZ[:, c0:c1],
                                lhsT=ytsb[:, j * W + u * P: j * W + (u + 1) * P],
                                rhs=bandb[:, c0 - j * P + 32: c1 - j * P + 32],
                                start=False, stop=(j == NT - 1))
                        # t = clamp01(z) * D + 1 - D   (fused custom DVE op)
                        nc.vector._custom_dve(
                            RAIN_OP, out=tcat[u][:, c * W:(c + 1) * W],
                            in0=psZ[:], in1=D_t[u][:],
                        )
                        if last:
                            # per-(c,u) salt/pepper + store: shrinks the tail
                            nc.vector._custom_dve(
                                SP_OP, out=ocat[u][:, c * W:(c + 1) * W],
                                in0=ncat[:, u * CW + c * W: u * CW + (c + 1) * W],
                                in1=tcat[u][:, c * W:(c + 1) * W],
                            )
                            nc.sync.dma_start(
                                out=out_d[b, u][:, c * W:(c + 1) * W],
                                in_=ocat[u][:, c * W:(c + 1) * W])
                        elif c == C - 1:
                            # whole-u salt/pepper + store
                            nc.vector._custom_dve(
                                SP_OP, out=ocat[u][:],
                                in0=ncat[:, u * CW:(u + 1) * CW],
                                in1=tcat[u][:],
                            )
                            nc.sync.dma_start(out=out_d[b, u], in_=ocat[u][:])
    nc.finalize()
    return nc


def _get_module():
    if "nc" not in _CACHE:
        _CACHE["nc"] = _build_module()
    return _CACHE["nc"]


def kernel(**inputs):
    x = np.asarray(inputs["x"], dtype=np.float32)
    noise = np.asarray(inputs["noise_u"], dtype=np.float32)
    xp, m, pb, rg = _host_params(
        x, np.asarray(inputs["sigma_u"]), np.asarray(inputs["glare_u"]),
        np.asarray(inputs["occ_u"]), np.asarray(inputs["rain_u"]),
        np.asarray(inputs["rain_n_u"]), np.asarray(inputs["rain_alpha_u"]),
        noise, np.asarray(inputs["noise_amt_u"]),
        np.asarray(inputs["apply_flags"]),
    )

    from concourse.bass_utils import run_bass_kernel_spmd

    nc = _get_module()
    in_maps = []
    for i in range(NCORES):
        s = slice(i * BPC, (i + 1) * BPC)
        in_maps.append({
            "x": np.ascontiguousarray(xp[s]),
            "m": np.ascontiguousarray(m[s]),
            "pb": np.ascontiguousarray(pb[s]),
            "rg": np.ascontiguousarray(rg[s]),
        })
    import os
    trace_env = os.environ.get("CPERT_TRACE", "")
    kw = {}
    if trace_env:
        kw["trace"] = True
        kw["trace_cores"] = [int(c) for c in trace_env.split(",")]
    res = run_bass_kernel_spmd(nc, in_maps, list(range(NCORES)), **kw)
    if trace_env:
        _CACHE["last_results"] = res
    o = np.concatenate([r["out"] for r in res.results], axis=0)  # [B,NT,P,CW]
    o = o.reshape(B, NT, P, C, W).transpose(0, 3, 1, 2, 4).reshape(B, C, H, W)
    return np.ascontiguousarray(o).astype(np.float32)


# revision 12
# speedup vs baseline: 1.0259x; 1.0259x over previous
"""CompositePerturbation Trainium2 kernel (v4 — pipelined, packed params).

Pipeline per sample (batch sharded 4-per-core across 8 cores):
  1. Separable 25-tap blur as two banded-matmul passes with the BAND as the
     moving operand, streaming only its ~146 nonzero columns per 128-block:
       pass1: Y^T[w,h] = sum_k X_k[h_in,w]^T-free  x  bandV[h_in, h_out-narrow]
       pass2: Z[h,w']  = sum_j Y^T_j[w,h]-cols     x  bandH[w,  w'-narrow]
     All matmuls are narrow (32-aligned ranges): the k=0 matmul carries
     start=True, marking the whole 2KB zero-region pending; later matmuls
     accumulate into already-written bytes and overwrite still-pending ones
     (per-byte has_written semantics, verified on HW). x/band are bf16.
  2. Glare + occlusion: single K=2 bf16 rank-2 matmul (gy|-BIG*ro x
     gx|co) injected first (full width) into each pass-2 PSUM tile.
  3. Rain: count = K=60 matmul of exact 0/1 fp8 streak masks; decay
     D = exp(L*count) on ScalarE with per-partition scale L (fp32 exact).
  4. t = clamp01(z)*D + 1 - D  (custom DVE op, PSUM source) -> bf16.
  5. Salt/pepper as a host-precomputed exact trit mask m in {0,1,2} (fp8):
     out = min(max(t, m-1), m) as a single fused custom DVE op.

v4 vs v3:
  - params packed: band+scal one bf16 DMA, rn+gvt one fp8 DMA (bitcast APs)
  - x(c0) DMA issued first so pass1 starts ~6us earlier
  - junk matmuls dropped; rain matmuls sit between pass1(c0) and pass2(c0)
  - salt/pepper per-u tile (per-(c,u) on the last sample to shrink the tail)
  - 4 pass-2 PSUM banks
"""

import numpy as np

B, C, H, W = 32, 3, 512, 512
NCORES = 8
BPC = B // NCORES  # samples per core
KS = 25
HALF = KS // 2  # 12
P = 128
NT = H // P  # 4 partition tiles per image
BIGNEG = 100.0
CW = 3 * W  # 1536
BW = 192  # slim band tile width (covers offsets -32..160 of each k-block)

_CACHE = {}

# Narrow output ranges per contraction block k, superset of the true
# support [k*128-12, k*128+140). Offsets MUST stay 32-aligned: unaligned
# partial-PSUM matmul writes hard-wedge the device (NRT_EXEC_UNIT_UNRECOVERABLE).
def _nrange(k):
    return max(0, k * P - 32), min(512, k * P + P + 32)


def _host_params(x, sigma_u, glare_u, occ_u, rain_u, rain_n_u, rain_alpha_u,
                 noise_u, noise_amt_u, apply_flags):
    import ml_dtypes
    f32 = np.float32
    bf16 = ml_dtypes.bfloat16
    fp8 = ml_dtypes.float8_e4m3
    flags = apply_flags.astype(np.int64)

    # ---- blur band tensor [B, 128, 192]: band[r, t] = f(t - 32 - r) ----
    sigma = 1.0 + 3.0 * sigma_u.astype(np.float64)
    coords = np.arange(KS, dtype=np.float64) - HALF
    g = np.exp(-coords[None, :] ** 2 / (2.0 * sigma[:, None] ** 2))
    g = (g / g.sum(axis=1, keepdims=True)).astype(f32)  # [B, 25]
    band = np.zeros((B, P, BW), dtype=f32)
    r = np.arange(P)[:, None]
    t = np.arange(BW)[None, :]
    d = t - 32 - r
    inband = np.abs(d) <= HALF
    for b in range(B):
        if flags[b, 0] > 0:
            vals = np.zeros((P, BW), dtype=f32)
            vals[inband] = g[b][(d[inband] + HALF)]
            band[b] = vals
        else:
            band[b] = (d == 0).astype(f32)
    band = band.astype(bf16)

    # ---- glare/occl rank-2 tensor gvt [B, 2, 1024] ----
    yy = np.arange(H, dtype=f32)
    xx = np.arange(W, dtype=f32)
    inten = 0.4 + 0.5 * glare_u[:, 0]
    rx = (0.1 + 0.25 * glare_u[:, 1]) * W / 2
    ry = (0.1 + 0.25 * glare_u[:, 2]) * H / 2
    cx = (0.2 + 0.6 * glare_u[:, 3]) * W
    cy = (0.2 + 0.6 * glare_u[:, 4]) * H
    gy = np.exp(-((yy[None, :] - cy[:, None]) / ry[:, None]) ** 2)
    gx = np.exp(-((xx[None, :] - cx[:, None]) / rx[:, None]) ** 2)
    gy = gy * inten[:, None] * (flags[:, 1] > 0)[:, None]

    ph = np.floor(H * (0.1 + 0.3 * occ_u[:, 0]))
    pw = np.floor(W * (0.1 + 0.3 * occ_u[:, 1]))
    y0 = np.floor(occ_u[:, 2] * (H - ph))
    x0 = np.floor(occ_u[:, 3] * (W - pw))
    ro = ((yy[None, :] >= y0[:, None]) & (yy[None, :] < (y0 + ph)[:, None]))
    co = ((xx[None, :] >= x0[:, None]) & (xx[None, :] < (x0 + pw)[:, None]))
    ro = ro & (flags[:, 2] > 0)[:, None]

    gvt = np.zeros((B, 2, 1024), dtype=f32)
    gvt[:, 0, :512] = gy
    gvt[:, 1, :512] = (-BIGNEG) * ro
    gvt[:, 0, 512:] = gx
    gvt[:, 1, 512:] = co
    gvt = gvt.astype(bf16)

    # ---- rain streak masks (exact 0/1 in fp8), L via activation scale ----
    S = rain_u.shape[1]
    n = np.floor(20.0 + 41.0 * rain_n_u)
    a = 0.15 + 0.35 * rain_alpha_u
    L = np.where(flags[:, 3] > 0, np.log(1.0 - a), 0.0).astype(f32)
    xc = np.floor(rain_u[:, :, 0] * W)
    y0s = np.floor(rain_u[:, :, 1] * (H // 2))
    y1s = (H // 2) + np.floor(rain_u[:, :, 2] * (H // 2))
    hh = np.arange(H, dtype=f32)[None, None, :]
    ww = np.arange(W, dtype=f32)[None, None, :]
    rowm = ((hh >= y0s[:, :, None]) & (hh < y1s[:, :, None]))
    colm = ((ww >= xc[:, :, None] - 1) & (ww <= xc[:, :, None]))
    active = (np.arange(S)[None, :] < n[:, None])
    rn = np.concatenate([rowm & active[:, :, None], colm], axis=2)
    rn = rn.astype(fp8)  # [B, 60, 1024]

    scal = np.zeros((B, P, 2), dtype=f32)
    scal[:, :, 0] = L[:, None]

    # ---- packed param tensors ----
    # pb [B, 128, 196] bf16: cols 0:192 band, cols 192:196 scal (f32 bytes)
    pb = np.zeros((B, P, 196), dtype=bf16)
    pb[:, :, 0:BW] = band
    pb[:, :, BW:BW + 4] = scal.view(bf16)
    # rg [B, 66, 2048] fp8: rows 0:60 cols 0:1024 rn, rows 64:66 gvt bytes
    # (gvt must sit at a 32-aligned partition for matmul tile_position)
    rg = np.zeros((B, 66, 2048), dtype=fp8)
    rg[:, 0:60, 0:1024] = rn
    rg[:, 64:66, :] = gvt.view(fp8)

    # ---- salt/pepper trit mask m in {0,1,2}, exact fp32 compares ----
    amount = (0.01 + 0.07 * noise_amt_u)[:, None, None, None]
    f4 = (flags[:, 4] > 0)[:, None, None, None]
    lo = np.where(f4, amount / 2, 0.0)
    hi = np.where(f4, 1.0 - amount / 2, 2.0)
    trit = (1.0 + (noise_u > hi) - (noise_u < lo)).astype(fp8)
    m = np.ascontiguousarray(
        trit.reshape(B, C, NT, P, W).transpose(0, 3, 2, 1, 4)
    ).reshape(B, P, NT * CW)

    # ---- x packed [B, C, 128, NT*W] bf16 (channel-contiguous) ----
    xp = np.ascontiguousarray(
        x.astype(bf16).reshape(B, C, NT, P, W).transpose(0, 1, 3, 2, 4)
    ).reshape(B, C, P, NT * W)

    return xp, m, pb, rg


def _register_dve_ops():
    """Register the fused rain/salt-pepper custom DVE ops.

    CPERT_RAIN: out = clamp01(in0) * in1 + 1 - in1
      (in0 = blur+glare-BIG*occl PSUM, in1 = rain decay D)
    CPERT_SP2:  out = min(max(in1, in0 - 1), in0)
      (in0 = trit mask m, in1 = t)
    """
    from concourse import dve_ops
    from concourse.dve_spec import (
        Spec, Src0, Src1, Zero, One, maxx, minn, lower, _has_src1,
    )
    from concourse.dve_uop import DveOpSpec
    import numpy as np

    if "CPERT_RAIN_ANT" in dve_ops._SUB_OPCODE_FOR_NAME:
        return (dve_ops._BY_NAME_CPERT["CPERT_RAIN_ANT"],
                dve_ops._BY_NAME_CPERT["CPERT_SP2_ANT"])

    def make(name, spec):
        row = dve_ops._CUSTOM_DVE_ROW_BASE + len(dve_ops.OPS)
        assert row < 0x20
        shas = {}
        for ver in ("v3", "v4"):
            tmp = DveOpSpec(name=name, opcode=row, uops=lower(spec, ver=ver),
                            rd1_en=_has_src1(spec))
            shas[ver] = tmp.sha(ver)
        op = dve_ops.DveOp(name, spec, False, shas)
        dve_ops._SUB_OPCODE_FOR_NAME[name] = row
        dve_ops.OPS.append(op)
        dve_ops.CUSTOM_DVE_SPECS[name] = spec
        return op

    rain_spec = Spec(
        body=maxx(minn(Src0, One), Zero) * Src1 + One - Src1,
        reference=lambda in0, in1, s0, s1, imm2: (
            np.clip(in0, 0.0, 1.0).astype(np.float32) * in1 + 1.0 - in1
        ).astype(np.float32),
    )
    sp_spec = Spec(
        body=minn(maxx(Src1, Src0 - One), Src0),
        reference=lambda in0, in1, s0, s1, imm2: np.minimum(
            np.maximum(in1, in0 - 1.0), in0).astype(np.float32),
    )
    rain_op = make("CPERT_RAIN_ANT", rain_spec)
    sp_op = make("CPERT_SP2_ANT", sp_spec)
    dve_ops._BY_NAME_CPERT = {"CPERT_RAIN_ANT": rain_op,
                              "CPERT_SP2_ANT": sp_op}
    return rain_op, sp_op


def _build_module():
    import concourse.bacc as bacc
    import concourse.mybir as mybir
    from concourse.tile import TileContext

    f32 = mybir.dt.float32
    bf16 = mybir.dt.bfloat16
    fp8 = mybir.dt.float8e4
    AF = mybir.ActivationFunctionType

    RAIN_OP, SP_OP = _register_dve_ops()

    nc = bacc.Bacc("TRN2", target_bir_lowering=False, debug=False,
                   num_devices=NCORES)
    x_d = nc.declare_dram_parameter("x", [BPC, C, P, NT * W], bf16, isOutput=False)
    m_d = nc.declare_dram_parameter("m", [BPC, P, NT * CW], fp8, isOutput=False)
    pb_d = nc.declare_dram_parameter("pb", [BPC, P, 196], bf16, isOutput=False)
    rg_d = nc.declare_dram_parameter("rg", [BPC, 66, 2048], fp8, isOutput=False)
    out_d = nc.declare_dram_parameter("out", [BPC, NT, P, CW], bf16, isOutput=True)

    with TileContext(nc) as tc:
        with (
            tc.tile_pool(name="params", bufs=2) as ppool,
            tc.tile_pool(name="xin", bufs=5) as xpool,
            tc.tile_pool(name="ytsb", bufs=2) as ytpool,
            tc.tile_pool(name="tcat", bufs=8) as tpool,
            tc.tile_pool(name="ncat", bufs=2) as npool,
            tc.tile_pool(name="dd", bufs=8) as dpool,
            tc.tile_pool(name="oc", bufs=8) as opool,
            tc.tile_pool(name="yps", bufs=2, space="PSUM") as ypsum,
            tc.tile_pool(name="zps", bufs=4, space="PSUM") as zpsum,
            tc.tile_pool(name="rps", bufs=2, space="PSUM") as rpsum,
        ):
            warmed = False
            for b in range(BPC):
                last = (b == BPC - 1)
                # x(c0) first: it gates pass 1 of this sample
                xq = []
                xt0 = xpool.tile([P, NT * W], bf16, tag="x", name=f"x{b}_0")
                nc.sync.dma_start(out=xt0[:], in_=x_d[b, 0])
                xq.append(xt0)

                pbt = ppool.tile([P, 196], bf16, tag="pb")
                nc.sync.dma_start(out=pbt[:], in_=pb_d[b])
                bandb = pbt[:, 0:BW]
                sc = pbt[:, BW:BW + 4].bitcast(f32)

                rgt = ppool.tile([66, 2048], fp8, tag="rg")
                nc.sync.dma_start(out=rgt[:], in_=rg_d[b])
                rn = rgt[0:60, 0:1024]
                gvt = rgt[64:66, :].bitcast(bf16)

                ncat = npool.tile([P, NT * CW], fp8, tag="n")
                nc.sync.dma_start(out=ncat[:], in_=m_d[b])

                for c in range(1, C):
                    xt = xpool.tile([P, NT * W], bf16, tag="x", name=f"x{b}_{c}")
                    nc.sync.dma_start(out=xt[:], in_=x_d[b, c])
                    xq.append(xt)

                if not warmed:
                    # dummy Exp absorbs the ACT table load + bias-const dep
                    warm = ppool.tile([P, 2], f32, tag="warm")
                    nc.scalar.activation(warm[:, 0:1], sc[:, 1:2], AF.Exp)
                    warmed = True

                # ---- rain decay D[u] = exp(L * count) ----
                D_t = []
                for u in range(NT):
                    psA = rpsum.tile([P, W], f32, tag="psA")
                    nc.tensor.matmul(psA[:],
                                     lhsT=rn[0:60, u * P:(u + 1) * P],
                                     rhs=rn[0:60, 512:1024],
                                     start=True, stop=True)
                    dt_ = dpool.tile([P, W], f32, tag="D", name=f"D{b}_{u}")
                    nc.scalar.activation(dt_[:], psA[:], AF.Exp,
                                         bias=sc[:, 1:2], scale=sc[:, 0:1])
                    D_t.append(dt_)

                tcat = [tpool.tile([P, CW], bf16, tag="t", name=f"tc{b}_{u}")
                        for u in range(NT)]
                ocat = [opool.tile([P, CW], bf16, tag="o", name=f"oc{b}_{u}")
                        for u in range(NT)]

                for c in range(C):
                    # ---- pass 1: Y^T tiles, band moving w/ narrow columns ----
                    ytsb = ytpool.tile([P, NT * W], bf16, tag="yt")
                    for i in range(NT):
                        psY = ypsum.tile([P, W], f32, tag="psY")
                        # all-narrow: k=0 start=True marks the bank pending;
                        # later matmuls accumulate written bytes, overwrite
                        # still-pending ones (per-byte has_written)
                        for k in range(NT):
                            c0, c1 = _nrange(k)
                            nc.tensor.matmul(
                                psY[:, c0:c1],
                                lhsT=xq[c][:, k * W + i * P: k * W + (i + 1) * P],
                                rhs=bandb[:, c0 - k * P + 32: c1 - k * P + 32],
                                start=(k == 0), stop=(k == NT - 1))
                        nc.scalar.copy(ytsb[:, i * W:(i + 1) * W], psY[:])

                    # ---- pass 2 + glare/occl inject + rain + salt/pepper ----
                    for u in range(NT):
                        psZ = zpsum.tile([P, W], f32, tag="psZ")
                        nc.tensor.matmul(psZ[:], lhsT=gvt[:, u * P:(u + 1) * P],
                                         rhs=gvt[:, 512:1024],
                                         start=True, stop=False)
                        for j in range(NT):
                            c0, c1 = _nrange(j)
                            nc.tensor.matmul(
                                psZ[:, c0:c1],
                                lhsT=ytsb[:, j * W + u * P: j * W + (u + 1) * P],
                                rhs=bandb[:, c0 - j * P + 32: c1 - j * P + 32],
                                start=False, stop=(j == NT - 1))
                        # t = clamp01(z) * D + 1 - D   (fused custom DVE op)
                        nc.vector._custom_dve(
                            RAIN_OP, out=tcat[u][:, c * W:(c + 1) * W],
                            in0=psZ[:], in1=D_t[u][:],
                        )
                        if last:
                            # per-(c,u) salt/pepper + store: shrinks the tail
                            nc.vector._custom_dve(
                                SP_OP, out=ocat[u][:, c * W:(c + 1) * W],
                                in0=ncat[:, u * CW + c * W: u * CW + (c + 1) * W],
                                in1=tcat[u][:, c * W:(c + 1) * W],
                            )
                            nc.sync.dma_start(
                                out=out_d[b, u][:, c * W:(c + 1) * W],
                                in_=ocat[u][:, c * W:(c + 1) * W])
                        elif c == C - 1:
                            # whole-u salt/pepper + store
                            nc.vector._custom_dve(
                                SP_OP, out=ocat[u][:],
                                in0=ncat[:, u * CW:(u + 1) * CW],
                                in1=tcat[u][:],
                            )
                            nc.sync.dma_start(out=out_d[b, u], in_=ocat[u][:])
    nc.finalize()
    return nc


def _get_module():
    if "nc" not in _CACHE:
        _CACHE["nc"] = _build_module()
    return _CACHE["nc"]


def kernel(**inputs):
    x = np.asarray(inputs["x"], dtype=np.float32)
    noise = np.asarray(inputs["noise_u"], dtype=np.float32)
    xp, m, pb, rg = _host_params(
        x, np.asarray(inputs["sigma_u"]), np.asarray(inputs["glare_u"]),
        np.asarray(inputs["occ_u"]), np.asarray(inputs["rain_u"]),
        np.asarray(inputs["rain_n_u"]), np.asarray(inputs["rain_alpha_u"]),
        noise, np.asarray(inputs["noise_amt_u"]),
        np.asarray(inputs["apply_flags"]),
    )

    from concourse.bass_utils import run_bass_kernel_spmd

    nc = _get_module()
    in_maps = []
    for i in range(NCORES):
        s = slice(i * BPC, (i + 1) * BPC)
        in_maps.append({
            "x": np.ascontiguousarray(xp[s]),
            "m": np.ascontiguousarray(m[s]),
            "pb": np.ascontiguousarray(pb[s]),
            "rg": np.ascontiguousarray(rg[s]),
        })
    import os
    trace_env = os.environ.get("CPERT_TRACE", "")
    kw = {}
    if trace_env:
        kw["trace"] = True
        kw["trace_cores"] = [int(c) for c in trace_env.split(",")]
    res = run_bass_kernel_spmd(nc, in_maps, list(range(NCORES)), **kw)
    if trace_env:
        _CACHE["last_results"] = res
    o = np.concatenate([r["out"] for r in res.results], axis=0)  # [B,NT,P,CW]
    o = o.reshape(B, NT, P, C, W).transpose(0, 3, 1, 2, 4).reshape(B, C, H, W)
    return np.ascontiguousarray(o).astype(np.float32)
